# revision 1
# baseline (speedup 1.0000x reference)
"""Multi-head attention (B=4, S=2048, D=1024, H=16) on 8 trn2 NeuronCores.

Sharding: core c = 2*b + g handles batch b, head-group g (8 heads, 512 dims).
Q/K/V projections are column-sharded (Megatron), Wo row-sharded; the Wo
partial sums for the two head-groups of each batch are reduced host-side.

Device layout notes:
  - All activations live in "transposed" [feature, seq] layout so every
    matmul has its contraction dim on SBUF partitions.  Host pre-transposes.
  - Matmul operands are FP32R (full PE rate for free dim >= 256, ~1e-4 rel
    precision).  PSUM stays fp32.  DMA'd operands are staged raw and cast to
    f32r by GPSIMD; engine-produced operands are written as f32r directly.
  - Scores are computed as S^T [k, q] tiles; softmax denominators come from
    an extra ones-column appended to V (PV matmul computes [O^T; rowsum]).
  - PV matmuls are emitted one exp-group behind the S matmuls so the
    in-order PE queue never waits on a fresh ACT exp.
  - Causal: K/V projections are interleaved with attention per 512-row
    block (attention for q-block qb only needs k-chunks <= 4*qb+3), and
    upper-triangular score blocks are skipped entirely; diagonal blocks are
    zeroed multiplicatively post-exp.
  - Input loads ride the SP DMA queue; output stores ride the ACT queue so
    stores never head-of-line-block the next block's loads.
"""

import numpy as np

import concourse.mybir as mybir
import concourse.tile as tile
from concourse import bacc
from concourse.bass_utils import run_bass_kernel_spmd

F32 = mybir.dt.float32
F32R = mybir.dt.float32r
B, S, D, H, DK = 4, 2048, 1024, 16, 64
DL = 512  # dims per head-group (8 heads * 64)
NH = 8  # heads per core
NHP = 4  # head pairs per core
NDI = D // 128  # 8   d_model chunks
NSC = S // 128  # 16  seq chunks (k side)
NSB = S // 512  # 4   seq blocks (q side)
NDC = DL // 128  # 4  local-dim chunks
EXP_GRP = 2  # k-chunks per exp instruction
NEG = -1.0e9


def _load_cast(env, dst3, dram_ap, chunk, nchunks, base):
    """DMA f32 chunks into raw staging tiles, GPSIMD-cast into f32r dst."""
    nc = env["nc"]
    for i in range(nchunks):
        raw = env["raws"].tile([128, 512], F32, tag="raw", name=f"{base}{i}")
        nc.sync.dma_start(
            out=raw[:, :chunk], in_=dram_ap[i * 128 : (i + 1) * 128, :]
        )
        nc.gpsimd.tensor_copy(dst3[:, i, :], raw[:, :chunk])


def _emit_kproj(env, sb):
    nc = env["nc"]
    ssl = slice(sb * 512, (sb + 1) * 512)
    kt = env["stream"].tile([128, NDI, 512], F32R, tag="qkstream", name="kt")
    _load_cast(env, kt, env["kT_d"].ap()[:, ssl], 512, NDI, "ktraw")
    for dc in range(NDC):
        ps = env["pss"].tile([128, 512], F32, tag="s", name="ppk")
        for di in range(NDI):
            nc.tensor.matmul(
                ps[:],
                env["wk_r"][:, di, dc * 128 : (dc + 1) * 128],
                kt[:, di, :],
                start=(di == 0),
                stop=(di == NDI - 1),
            )
        nc.vector.tensor_scalar_add(
            env["Ksb"][:, dc, ssl], ps[:], env["bk_sb"][:, dc : dc + 1]
        )


def _emit_vproj(env, sc):
    nc = env["nc"]
    vtr = env["vstream"].tile([128, NDI, 128], F32R, tag="vtr", name="vtr")
    _load_cast(
        env, vtr, env["vT_d"].ap()[:, sc * 128 : (sc + 1) * 128], 128, NDI, "vtraw"
    )
    ps = env["pss"].tile([128, DL], F32, tag="s", name="ppv")
    for di in range(NDI):
        nc.tensor.matmul(
            ps[:],
            vtr[:, di, :],
            env["wv_r"][:, di, :],
            start=(di == 0),
            stop=(di == NDI - 1),
        )
    vt3 = env["vts"][sc][:].rearrange("p (h c) -> p h c", h=NH)
    nc.vector.tensor_add(
        vt3[:, :, 0:64],
        ps[:].rearrange("p (h c) -> p h c", h=NH),
        env["bv_sb"][:].rearrange("p (h c) -> p h c", h=NH),
    )
    nc.vector.tensor_copy(vt3[:, :, 64:65], env["ones_sb"][:].unsqueeze(2))


def _emit_s_exp(env, qb, hp, grp, mq_sb, skip_mask=False, skip_exp=False):
    nc = env["nc"]
    variant = env["variant"]
    pss, ep = env["pss"], env["ep"]
    Ksb, Qblk, mt_sb = env["Ksb"], env["Qblk"], env["mt_sb"]
    ng = len(grp)
    ps_s = {
        0: pss.tile([128, EXP_GRP * 512], F32, tag="s", name="pssA"),
        1: pss.tile([128, EXP_GRP * 512], F32, tag="s", name="pssB"),
    }
    for hb, (p0, p1) in enumerate(((0, 64), (64, 128))):
        for j, kc in enumerate(grp):
            nc.tensor.matmul(
                ps_s[hb][:, j * 512 : (j + 1) * 512],
                Ksb[p0:p1, hp, kc * 128 : (kc + 1) * 128],
                Qblk[p0:p1, hp, :],
                start=True,
                stop=True,
            )
    if variant == "general" and not skip_mask:
        for j, kc in enumerate(grp):
            psl = slice(j * 512, (j + 1) * 512)
            for hb in range(2):
                nc.vector.tensor_add(
                    ps_s[hb][:, psl], ps_s[hb][:, psl], mq_sb[:, kc, :]
                )
    if skip_exp:
        return None
    es = {}
    for hb in range(2):
        et = ep.tile([128, EXP_GRP * 512], F32R, tag="e", name=f"e{hb}")
        nc.scalar.activation(
            et[:, : ng * 512],
            ps_s[hb][:, : ng * 512],
            mybir.ActivationFunctionType.Exp,
            scale=1.0 / np.sqrt(DK),
        )
        es[hb] = et
    # causal: multiplicative zeroing post-exp on SBUF (off the ACT chain)
    if variant == "causal" and not skip_mask:
        for j, kc in enumerate(grp):
            if kc >= 4 * qb:
                psl = slice(j * 512, (j + 1) * 512)
                for hb in range(2):
                    nc.vector.tensor_mul(
                        es[hb][:, psl], es[hb][:, psl], mt_sb[:, kc - 4 * qb, :]
                    )
    return es


def _emit_pv(env, hp, grp, first, last, es, ps_o):
    nc = env["nc"]
    vts = env["vts"]
    for hb in range(2):
        for j, kc in enumerate(grp):
            h = 2 * hp + hb
            nc.tensor.matmul(
                ps_o[hb][:],
                vts[kc][:, h * 65 : (h + 1) * 65],
                es[hb][:, j * 512 : (j + 1) * 512],
                start=(kc == first),
                stop=(kc == last),
            )


def _emit_qblock(env, qb):
    nc = env["nc"]
    variant = env["variant"]
    skip = env["skip"]
    qsl = slice(qb * 512, (qb + 1) * 512)

    # Q projection for this q block
    qt = env["stream"].tile([128, NDI, 512], F32R, tag="qkstream", name="qt")
    _load_cast(env, qt, env["qT_d"].ap()[:, qsl], 512, NDI, "qtraw")
    Qblk = env["qbp"].tile([128, NDC, 512], F32R, tag="Qblk", name="Qblk")
    env["Qblk"] = Qblk
    for dc in range(NDC):
        ps = env["pss"].tile([128, 512], F32, tag="s", name="ppq")
        for di in range(NDI):
            nc.tensor.matmul(
                ps[:],
                env["wq_r"][:, di, dc * 128 : (dc + 1) * 128],
                qt[:, di, :],
                start=(di == 0),
                stop=(di == NDI - 1),
            )
        nc.vector.tensor_scalar_add(
            Qblk[:, dc, :], ps[:], env["bq_sb"][:, dc : dc + 1]
        )

    mq_sb = None
    if variant == "general":
        mq_sb = env["mqp"].tile([128, NSC, 512], F32, tag="mq", name="mq")
        nc.sync.dma_start(
            out=mq_sb[:],
            in_=env["mT_d"].ap()[:, qsl].rearrange("(c p) q -> p c q", p=128),
        )

    kept = env["kept_kcs"](qb)
    groups = [kept[i : i + EXP_GRP] for i in range(0, len(kept), EXP_GRP)]
    first, last = kept[0], kept[-1]
    skip_pv = "pv" in skip or "exp" in skip
    skip_mask = "nomask" in skip or "exp" in skip
    skip_exp = "exp" in skip
    Xblk = None
    if not ({"pv", "exp", "attn"} & set(skip)):
        Xblk = env["xbp"].tile([128, NDC, 512], F32R, tag="Xblk", name="Xblk")

    def _normalize(hp, ps_o):
        for hb, (p0, p1) in enumerate(((0, 64), (64, 128))):
            r = env["rp"].tile([1, 512], F32, tag="r", name=f"r{hb}")
            rb = env["rp"].tile([64, 512], F32, tag="rb", name=f"rb{hb}")
            nc.vector.reciprocal(r[:], ps_o[hb][64:65, :])
            nc.gpsimd.partition_broadcast(rb[:], r[0:1, :])
            nc.vector.tensor_mul(Xblk[p0:p1, hp, :], ps_o[hb][0:64, :], rb[:])

    if "attn" not in skip:
        units = [(hp, gi, grp) for hp in range(NHP)
                 for gi, grp in enumerate(groups)]
        ps_os = {}
        pend = None
        for hp, gi, grp in units:
            es = _emit_s_exp(env, qb, hp, grp, mq_sb,
                             skip_mask=skip_mask, skip_exp=skip_exp)
            if pend is not None and not skip_pv:
                phw, pgi, pgrp, pes = pend
                if pgi == 0:
                    ps_os[phw] = {
                        0: env["pso"].tile([65, 512], F32, tag="oA", name="psoA"),
                        1: env["pso"].tile([65, 512], F32, tag="oB", name="psoB"),
                    }
                _emit_pv(env, phw, pgrp, first, last, pes, ps_os[phw])
                if pgi == len(groups) - 1:
                    _normalize(phw, ps_os.pop(phw))
            pend = (hp, gi, grp, es)
        if pend is not None and not skip_pv:
            phw, pgi, pgrp, pes = pend
            if pgi == 0:
                ps_os[phw] = {
                    0: env["pso"].tile([65, 512], F32, tag="oA", name="psoA"),
                    1: env["pso"].tile([65, 512], F32, tag="oB", name="psoB"),
                }
            _emit_pv(env, phw, pgrp, first, last, pes, ps_os[phw])
            _normalize(phw, ps_os.pop(phw))

    # ---- out projection for this q block (stores ride the ACT queue) ----
    if {"out", "pv", "attn", "exp"} & set(skip):
        return
    for ec in range(NDI):
        ps = env["pss"].tile([128, 512], F32, tag="s", name="ppc")
        for dl in range(NDC):
            nc.tensor.matmul(
                ps[:],
                env["wo_r"][:, dl, ec * 128 : (ec + 1) * 128],
                Xblk[:, dl, :],
                start=(dl == 0),
                stop=(dl == NDC - 1),
            )
        ot = env["osp"].tile([128, 512], F32, tag="ot", name="ot")
        nc.scalar.copy(ot[:], ps[:])
        nc.scalar.dma_start(
            out=env["outT_d"].ap()[ec * 128 : (ec + 1) * 128, qsl], in_=ot[:]
        )


def build_program(variant, reps=1, skip=()):
    """variant: 'causal' | 'ones' | 'general'; skip: timing-ablation flags."""
    assert variant in ("causal", "ones", "general")
    nc = bacc.Bacc("TRN2", target_bir_lowering=False, debug=False)

    qT_d = nc.dram_tensor("qT", [D, S], F32, kind="ExternalInput")
    kT_d = nc.dram_tensor("kT", [D, S], F32, kind="ExternalInput")
    vT_d = nc.dram_tensor("vT", [D, S], F32, kind="ExternalInput")
    wq_d = nc.dram_tensor("wq", [D, DL], F32, kind="ExternalInput")
    wk_d = nc.dram_tensor("wk", [D, DL], F32, kind="ExternalInput")
    wv_d = nc.dram_tensor("wv", [D, DL], F32, kind="ExternalInput")
    wo_d = nc.dram_tensor("wo", [DL, D], F32, kind="ExternalInput")
    bq_d = nc.dram_tensor("bq", [128, NDC], F32, kind="ExternalInput")
    bk_d = nc.dram_tensor("bk", [128, NDC], F32, kind="ExternalInput")
    bv_d = nc.dram_tensor("bv", [128, DL], F32, kind="ExternalInput")
    mt_d = mT_d = None
    if variant == "causal":
        # multiplicative 1/0 tiles for the 4 diagonal offsets [j, k, q]
        mt_d = nc.dram_tensor("maskt", [4, 128, 512], F32, kind="ExternalInput")
    elif variant == "general":
        # additive 0/-1e9, transposed [k, q]
        mT_d = nc.dram_tensor("maskT", [S, S], F32, kind="ExternalInput")
    outT_d = nc.dram_tensor("outT", [D, S], F32, kind="ExternalOutput")

    def kept_kcs(qb):
        return list(range(4 * qb + 4)) if variant == "causal" else list(range(NSC))

    with tile.TileContext(nc) as tc:
        for _rep in range(reps):
            with (
                tc.tile_pool(name="persist", bufs=1) as pers,
                tc.tile_pool(name="wts", bufs=1) as wts,
                tc.tile_pool(name="vt", bufs=1) as vtp,
                tc.tile_pool(name="stream", bufs=1) as stream,
                tc.tile_pool(name="vstream", bufs=1) as vstream,
                tc.tile_pool(name="raws", bufs=3) as raws,
                tc.tile_pool(name="qblk", bufs=1) as qbp,
                tc.tile_pool(name="xblk", bufs=1) as xbp,
                tc.tile_pool(name="epool", bufs=4) as ep,
                tc.tile_pool(name="rpool", bufs=1) as rp,
                tc.tile_pool(name="ostage", bufs=2) as osp,
                tc.tile_pool(name="mq", bufs=1) as mqp,
                tc.tile_pool(name="pss", bufs=3, space="PSUM") as pss,
                tc.tile_pool(name="pso", bufs=1, space="PSUM") as pso,
            ):
                env = dict(
                    nc=nc, variant=variant, skip=skip, kept_kcs=kept_kcs,
                    qT_d=qT_d, kT_d=kT_d, vT_d=vT_d, mT_d=mT_d, outT_d=outT_d,
                    stream=stream, vstream=vstream, raws=raws, qbp=qbp,
                    xbp=xbp, ep=ep, rp=rp, osp=osp, mqp=mqp, pss=pss, pso=pso,
                )
                # ---- constants ----
                bq_sb = pers.tile([128, NDC], F32, tag="bq", name="bq_sb")
                bk_sb = pers.tile([128, NDC], F32, tag="bk", name="bk_sb")
                bv_sb = pers.tile([128, DL], F32, tag="bv", name="bv_sb")
                ones_sb = pers.tile([128, NH], F32, tag="ones", name="ones_sb")
                nc.sync.dma_start(out=bq_sb[:], in_=bq_d.ap())
                nc.sync.dma_start(out=bk_sb[:], in_=bk_d.ap())
                nc.sync.dma_start(out=bv_sb[:], in_=bv_d.ap())
                nc.any.memset(ones_sb[:], 1.0)
                env.update(bq_sb=bq_sb, bk_sb=bk_sb, bv_sb=bv_sb, ones_sb=ones_sb)

                mt_sb = None
                if variant == "causal":
                    mt_sb = pers.tile([128, 4, 512], F32R, tag="mt", name="mt_sb")
                    for j in range(4):
                        raw = raws.tile([128, 512], F32, tag="raw", name=f"mtraw{j}")
                        nc.sync.dma_start(out=raw[:], in_=mt_d.ap()[j])
                        nc.vector.tensor_copy(mt_sb[:, j, :], raw[:])
                env["mt_sb"] = mt_sb

                for wname, wd in (("wk", wk_d), ("wv", wv_d), ("wq", wq_d)):
                    w_sb = wts.tile(
                        [128, NDI, DL], F32R, tag=wname, name=f"{wname}_sb"
                    )
                    _load_cast(env, w_sb, wd.ap(), DL, NDI, f"{wname}raw")
                    env[f"{wname}_r"] = w_sb[:]

                wo_sb = pers.tile([128, NDC, D], F32R, tag="wo", name="wo_sb")
                for dl in range(NDC):
                    for hh in range(2):
                        raw = raws.tile(
                            [128, 512], F32, tag="raw", name=f"woraw{dl}_{hh}"
                        )
                        nc.sync.dma_start(
                            out=raw[:],
                            in_=wo_d.ap()[
                                dl * 128 : (dl + 1) * 128, hh * 512 : (hh + 1) * 512
                            ],
                        )
                        nc.gpsimd.tensor_copy(
                            wo_sb[:, dl, hh * 512 : (hh + 1) * 512], raw[:]
                        )
                env["wo_r"] = wo_sb[:]

                Ksb = pers.tile([128, NDC, S], F32R, tag="Ksb", name="Ksb")
                vts = [
                    vtp.tile([128, NH * 65], F32R, tag=f"vt{sc}", name=f"vt{sc}")
                    for sc in range(NSC)
                ]
                env.update(Ksb=Ksb, vts=vts)

                if variant == "causal":
                    # interleave K/V projection blocks with attention blocks
                    for sb in range(NSB):
                        if "qk" not in skip:
                            _emit_kproj(env, sb)
                        if "v" not in skip:
                            for sc in range(4 * sb, 4 * sb + 4):
                                _emit_vproj(env, sc)
                        _emit_qblock(env, sb)
                else:
                    if "qk" not in skip:
                        for sb in range(NSB):
                            _emit_kproj(env, sb)
                    if "v" not in skip:
                        for sc in range(NSC):
                            _emit_vproj(env, sc)
                    for qb in range(NSB):
                        _emit_qblock(env, qb)
    nc.compile()
    return nc


# ---------------------------------------------------------------------------
# host side
# ---------------------------------------------------------------------------

_NC_CACHE = {}


def _get_program(variant, reps=1):
    key = (variant, reps)
    if key not in _NC_CACHE:
        _NC_CACHE[key] = build_program(variant, reps)
    return _NC_CACHE[key]


def detect_variant(mask):
    m = np.asarray(mask)
    if (m != 0).all():
        return "ones"
    tril = np.tril(np.ones((S, S), np.int8))
    for b in range(m.shape[0]):
        mb = (m[b] != 0).astype(np.int8)
        if not np.array_equal(mb, tril):
            return "general"
    return "causal"


def make_causal_mask_tiles():
    j = np.arange(4)[:, None, None]
    k = np.arange(128)[None, :, None]
    q = np.arange(512)[None, None, :]
    # multiplicative: 1 keep, 0 drop (applied to exp'd scores)
    return (q >= k + 128 * j).astype(np.float32)


def build_in_maps(query, key, value, mask, Wq, bq, Wk, bk, Wv, bv, Wo, bo, variant):
    query = np.asarray(query, np.float32)
    key = np.asarray(key, np.float32)
    value = np.asarray(value, np.float32)
    Wq, Wk, Wv, Wo = (np.asarray(w, np.float32) for w in (Wq, Wk, Wv, Wo))
    bq, bk, bv = (np.asarray(x, np.float32) for x in (bq, bk, bv))

    if variant == "causal":
        mtiles = make_causal_mask_tiles()

    in_maps = []
    for c in range(8):
        b, g = c // 2, c % 2
        gs = slice(g * DL, (g + 1) * DL)
        m = {
            "qT": np.ascontiguousarray(query[b].T),
            "kT": np.ascontiguousarray(key[b].T),
            "vT": np.ascontiguousarray(value[b].T),
            "wq": np.ascontiguousarray(Wq[gs].T),
            "wk": np.ascontiguousarray(Wk[gs].T),
            "wv": np.ascontiguousarray(Wv[gs].T),
            "wo": np.ascontiguousarray(Wo[:, gs].T),
            "bq": np.ascontiguousarray(bq[gs].reshape(NDC, 128).T),
            "bk": np.ascontiguousarray(bk[gs].reshape(NDC, 128).T),
            "bv": np.ascontiguousarray(np.broadcast_to(bv[gs], (128, DL))),
        }
        if variant == "causal":
            m["maskt"] = mtiles
        elif variant == "general":
            m["maskT"] = np.ascontiguousarray(
                np.where(np.asarray(mask[b]) != 0, 0.0, NEG).astype(np.float32).T
            )
        in_maps.append(m)
    return in_maps


def assemble_output(results, bo):
    bo = np.asarray(bo, np.float32)
    out = np.empty((B, S, D), np.float32)
    for b in range(B):
        acc = results[2 * b]["outT"] + results[2 * b + 1]["outT"]
        out[b] = acc.T + bo
    return out


def kernel(query, key, value, mask, Wq, bq, Wk, bk, Wv, bv, Wo, bo):
    variant = detect_variant(np.asarray(mask))
    in_maps = build_in_maps(
        query, key, value, mask, Wq, bq, Wk, bk, Wv, bv, Wo, bo, variant
    )
    nc = _get_program(variant)
    res = run_bass_kernel_spmd(nc, in_maps, core_ids=list(range(8)))
    return assemble_output(res.results, bo)



# revision 5
# speedup vs baseline: 1.9824x; 1.9824x over previous
"""Multi-head attention (B=4, S=2048, D=1024, H=16) on 8 trn2 NeuronCores.

Sharding: core c = 2*b + g handles batch b, head-group g (8 heads, 512 dims).
Q/K/V projections are column-sharded (Megatron), Wo row-sharded; the Wo
partial sums for the two head-groups of each batch are reduced host-side.

v2 design notes (vs the fp32r baseline):
  - All matmul operands are bf16 (host pre-casts inputs/weights).  Same PE
    cycles/column as fp32r but half the DMA + SBUF, no GPSIMD cast stage,
    no fp32r free>=256 rate cliff, and DVE 2x/4x modes on bf16 tiles.
  - PE p-state: the tensor engine only reaches 2.4 GHz after ~3us of
    gap-free execution, so the schedule is built to never starve the PE:
    each attention unit = one k-chunk with both head-halves packed in one
    [128, 1024] PSUM tile (2 banks), one exp per unit, PV lagged one unit
    behind scores, and projection matmul groups for the NEXT q-block (plus
    the PREVIOUS block's out-projection) interleaved as filler while ACT
    digests exp.
  - PSUM budget (8 banks): scores ring 2x[128,1024] (4) + PV accum
    2x[65,512] (2) + projection ring 2x[128,512] (2).
  - Causal: K/V projections interleaved per 512-row block, upper-triangle
    score chunks skipped, diagonal chunks zeroed multiplicatively post-exp
    (bf16 mask, DVE 4x).  Softmax denominators ride a ones-column in V.
"""

import numpy as np

import concourse.mybir as mybir
import concourse.tile as tile
from concourse import bacc
from concourse.bass_utils import run_bass_kernel_spmd

F32 = mybir.dt.float32
BF16 = mybir.dt.bfloat16
NPBF16 = mybir.dt.np(BF16)
B, S, D, H, DK = 4, 2048, 1024, 16, 64
DL = 512  # dims per head-group (8 heads * 64)
NH = 8  # heads per core
NHP = 4  # head pairs per core
NDI = D // 128  # 8   d_model chunks
NSC = S // 128  # 16  seq chunks (k side)
NSB = S // 512  # 4   seq blocks (q side)
NDC = DL // 128  # 4  local-dim chunks
NEG = -1.0e9


# ---------------------------------------------------------------------------
# device-side emission
# ---------------------------------------------------------------------------


def _emit_loads(env, qb):
    """Issue stream DMAs for block qb (kt/vt/qt), ring-2 buffered."""
    nc = env["nc"]
    ssl = slice(qb * 512, (qb + 1) * 512)
    for nm, dram in (("kt", env["kT_d"]), ("vt", env["vT_d"]), ("qt", env["qT_d"])):
        t = env["stream"].tile([128, NDI, 512], BF16, tag=f"{nm}s", name=f"{nm}{qb}")
        nc.sync.dma_start(
            out=t[:], in_=dram.ap()[:, ssl].rearrange("(c p) q -> p c q", p=128)
        )
        env[nm][qb] = t


def _kproj_group(env, sb, dc):
    nc = env["nc"]
    ssl = slice(sb * 512, (sb + 1) * 512)
    kt = env["kt"][sb]
    ps = env["ppp"].tile([128, 512], F32, tag="pp", name="ppk")
    for di in range(NDI):
        nc.tensor.matmul(
            ps[:],
            env["wk_r"][:, di, dc * 128 : (dc + 1) * 128],
            kt[:, di, :],
            start=(di == 0),
            stop=(di == NDI - 1),
        )
    nc.vector.tensor_scalar_add(
        env["Ksb"][:, dc, ssl], ps[:], env["bk_sb"][:, dc : dc + 1]
    )


def _qproj_group(env, qb, dc):
    nc = env["nc"]
    qt = env["qt"][qb]
    Qblk = env["Qblk"][qb]
    ps = env["ppp"].tile([128, 512], F32, tag="pp", name="ppq")
    for di in range(NDI):
        nc.tensor.matmul(
            ps[:],
            env["wq_r"][:, di, dc * 128 : (dc + 1) * 128],
            qt[:, di, :],
            start=(di == 0),
            stop=(di == NDI - 1),
        )
    nc.vector.tensor_scalar_add(Qblk[:, dc, :], ps[:], env["bq_sb"][:, dc : dc + 1])


def _vproj_group(env, sc):
    nc = env["nc"]
    vt = env["vt"][sc // 4]
    col = (sc % 4) * 128
    ps = env["ppp"].tile([128, DL], F32, tag="pp", name="ppv")
    for di in range(NDI):
        nc.tensor.matmul(
            ps[:],
            vt[:, di, col : col + 128],
            env["wv_r"][:, di, :],
            start=(di == 0),
            stop=(di == NDI - 1),
        )
    vt3 = env["vts"][sc][:].rearrange("p (h c) -> p h c", h=NH)
    nc.vector.tensor_add(
        vt3[:, :, 0:64],
        ps[:].rearrange("p (h c) -> p h c", h=NH),
        env["bv_sb"][:].rearrange("p (h c) -> p h c", h=NH),
    )
    nc.vector.tensor_copy(vt3[:, :, 64:65], env["ones_sb"][:].unsqueeze(2))


def _oproj_group(env, qb, ec):
    nc = env["nc"]
    qsl = slice(qb * 512, (qb + 1) * 512)
    Xblk = env["Xblk"][qb]
    ps = env["ppp"].tile([128, 512], F32, tag="pp", name="ppc")
    for dl in range(NDC):
        nc.tensor.matmul(
            ps[:],
            env["wo_r"][:, dl, ec * 128 : (ec + 1) * 128],
            Xblk[:, dl, :],
            start=(dl == 0),
            stop=(dl == NDC - 1),
        )
    ot = env["osp"].tile([128, 512], BF16, tag="ot", name="ot")
    nc.vector.tensor_copy(ot[:], ps[:])
    nc.scalar.dma_start(out=env["outT_d"].ap()[ec * 128 : (ec + 1) * 128, qsl], in_=ot[:])


def _emit_s_exp(env, qb, hp, kc, skip_mask=False, skip_exp=False):
    """Scores for one (head-pair, k-chunk): both halves in one PSUM tile,
    one exp, optional multiplicative mask.  Returns the bf16 es tile."""
    nc = env["nc"]
    variant = env["variant"]
    ps = env["pss"].tile([128, 1024], F32, tag="s", name="pss")
    for hb, (p0, p1) in enumerate(((0, 64), (64, 128))):
        nc.tensor.matmul(
            ps[:, hb * 512 : (hb + 1) * 512],
            env["Ksb"][p0:p1, hp, kc * 128 : (kc + 1) * 128],
            env["Qblk"][qb][p0:p1, hp, :],
            start=True,
            stop=True,
        )
    if skip_exp:
        return None
    et = env["ep"].tile([128, 1024], BF16, tag="e", name="et")
    nc.scalar.activation(
        et[:], ps[:], mybir.ActivationFunctionType.Exp, scale=1.0 / np.sqrt(DK)
    )
    if not skip_mask:
        if variant == "causal" and kc >= 4 * qb:
            nc.vector.tensor_mul(et[:], et[:], env["mt_sb"][:, kc - 4 * qb, :])
        elif variant == "general":
            for hb in range(2):
                nc.vector.tensor_mul(
                    et[:, hb * 512 : (hb + 1) * 512],
                    et[:, hb * 512 : (hb + 1) * 512],
                    env["mq_sb"][qb % 2][:, kc, :],
                )
    return et


def _emit_pv(env, hp, kc, first, last, es, ps_o):
    nc = env["nc"]
    for hb in range(2):
        h = 2 * hp + hb
        nc.tensor.matmul(
            ps_o[hb][:],
            env["vts"][kc][:, h * 65 : (h + 1) * 65],
            es[:, hb * 512 : (hb + 1) * 512],
            start=(kc == first),
            stop=(kc == last),
        )


def _emit_normalize(env, qb, hp, ps_o):
    nc = env["nc"]
    Xblk = env["Xblk"][qb]
    for hb, (p0, p1) in enumerate(((0, 64), (64, 128))):
        r = env["rp"].tile([1, 512], F32, tag="r", name=f"r{hb}")
        rb = env["rp"].tile([64, 512], F32, tag="rb", name=f"rb{hb}")
        nc.vector.reciprocal(r[:], ps_o[hb][64:65, :])
        nc.gpsimd.partition_broadcast(rb[:], r[0:1, :])
        nc.vector.tensor_mul(Xblk[p0:p1, hp, :], ps_o[hb][0:64, :], rb[:])


def _emit_attention(env, qb, filler):
    """Pipelined attention for block qb, draining `filler` (list of
    zero-arg closures emitting one PE matmul group each) at diagonal
    units and head-pair transitions."""
    nc = env["nc"]
    skip = env["skip"]
    kept = env["kept_kcs"](qb)
    first, last = kept[0], kept[-1]
    skip_pv = "pv" in skip or "exp" in skip
    skip_mask = "nomask" in skip or "exp" in skip
    skip_exp = "exp" in skip

    units = [(hp, kc) for hp in range(NHP) for kc in kept]
    fq = list(filler)
    fi = 0

    def pop_filler():
        nonlocal fi
        if fi < len(fq):
            fq[fi]()
            fi += 1

    # filler slots: before PV of diagonal units and at hp transitions
    def is_diag(kc):
        return env["variant"] == "causal" and kc >= 4 * qb

    pend = None
    ps_os = {}
    for hp, kc in units:
        es = _emit_s_exp(env, qb, hp, kc, skip_mask=skip_mask, skip_exp=skip_exp)
        if pend is not None:
            phw, pkc, pes = pend
            if is_diag(pkc) or pkc == last:
                pop_filler()
            if not skip_pv:
                if pkc == first:
                    ps_os[phw] = {
                        0: env["pso"].tile([65, 512], F32, tag="oA", name="psoA"),
                        1: env["pso"].tile([65, 512], F32, tag="oB", name="psoB"),
                    }
                _emit_pv(env, phw, pkc, first, last, pes, ps_os[phw])
                if pkc == last:
                    _emit_normalize(env, qb, phw, ps_os.pop(phw))
        pend = (hp, kc, es)
    if pend is not None and not skip_pv:
        phw, pkc, pes = pend
        pop_filler()
        if pkc == first:
            ps_os[phw] = {
                0: env["pso"].tile([65, 512], F32, tag="oA", name="psoA"),
                1: env["pso"].tile([65, 512], F32, tag="oB", name="psoB"),
            }
        _emit_pv(env, phw, pkc, first, last, pes, ps_os[phw])
        _emit_normalize(env, qb, phw, ps_os.pop(phw))
    # leftover filler
    while fi < len(fq):
        fq[fi]()
        fi += 1


def build_program(variant, reps=1, skip=()):
    """variant: 'causal' | 'ones' | 'general'; skip: timing-ablation flags."""
    assert variant in ("causal", "ones", "general")
    nc = bacc.Bacc("TRN2", target_bir_lowering=False, debug=False)

    qT_d = nc.dram_tensor("qT", [D, S], BF16, kind="ExternalInput")
    kT_d = nc.dram_tensor("kT", [D, S], BF16, kind="ExternalInput")
    vT_d = nc.dram_tensor("vT", [D, S], BF16, kind="ExternalInput")
    wq_d = nc.dram_tensor("wq", [D, DL], BF16, kind="ExternalInput")
    wk_d = nc.dram_tensor("wk", [D, DL], BF16, kind="ExternalInput")
    wv_d = nc.dram_tensor("wv", [D, DL], BF16, kind="ExternalInput")
    wo_d = nc.dram_tensor("wo", [DL, D], BF16, kind="ExternalInput")
    bq_d = nc.dram_tensor("bq", [128, NDC], F32, kind="ExternalInput")
    bk_d = nc.dram_tensor("bk", [128, NDC], F32, kind="ExternalInput")
    bv_d = nc.dram_tensor("bv", [128, DL], F32, kind="ExternalInput")
    mt_d = mT_d = None
    if variant == "causal":
        # multiplicative 1/0 tiles for the 4 diagonal offsets [j, k, 2*q]
        mt_d = nc.dram_tensor("maskt", [4, 128, 1024], BF16, kind="ExternalInput")
    elif variant == "general":
        # multiplicative 1/0, transposed [k, q]
        mT_d = nc.dram_tensor("maskT", [S, S], BF16, kind="ExternalInput")
    outT_d = nc.dram_tensor("outT", [D, S], BF16, kind="ExternalOutput")

    def kept_kcs(qb):
        return list(range(4 * qb + 4)) if variant == "causal" else list(range(NSC))

    with tile.TileContext(nc) as tc:
        for _rep in range(reps):
            with (
                tc.tile_pool(name="persist", bufs=1) as pers,
                tc.tile_pool(name="stream", bufs=2) as stream,
                tc.tile_pool(name="qblk", bufs=2) as qbp,
                tc.tile_pool(name="xblk", bufs=2) as xbp,
                tc.tile_pool(name="epool", bufs=4) as ep,
                tc.tile_pool(name="rpool", bufs=2) as rp,
                tc.tile_pool(name="ostage", bufs=3) as osp,
                tc.tile_pool(name="mq", bufs=2) as mqp,
                tc.tile_pool(name="pss", bufs=2, space="PSUM") as pss,
                tc.tile_pool(name="pso", bufs=1, space="PSUM") as pso,
                tc.tile_pool(name="ppool", bufs=2, space="PSUM") as ppp,
            ):
                env = dict(
                    nc=nc, variant=variant, skip=skip, kept_kcs=kept_kcs,
                    qT_d=qT_d, kT_d=kT_d, vT_d=vT_d, mT_d=mT_d, outT_d=outT_d,
                    stream=stream, ep=ep, rp=rp, osp=osp, ppp=ppp,
                    pss=pss, pso=pso,
                    kt={}, vt={}, qt={}, Qblk={}, Xblk={}, mq_sb={},
                )
                # ---- constants ----
                bq_sb = pers.tile([128, NDC], F32, tag="bq", name="bq_sb")
                bk_sb = pers.tile([128, NDC], F32, tag="bk", name="bk_sb")
                bv_sb = pers.tile([128, DL], F32, tag="bv", name="bv_sb")
                ones_sb = pers.tile([128, NH], BF16, tag="ones", name="ones_sb")
                nc.sync.dma_start(out=bq_sb[:], in_=bq_d.ap())
                nc.sync.dma_start(out=bk_sb[:], in_=bk_d.ap())
                nc.sync.dma_start(out=bv_sb[:], in_=bv_d.ap())
                nc.any.memset(ones_sb[:], 1.0)
                env.update(bq_sb=bq_sb, bk_sb=bk_sb, bv_sb=bv_sb, ones_sb=ones_sb)

                if variant == "causal":
                    mt_sb = pers.tile([128, 4, 1024], BF16, tag="mt", name="mt_sb")
                    nc.sync.dma_start(
                        out=mt_sb[:], in_=mt_d.ap().rearrange("j p q -> p j q")
                    )
                    env["mt_sb"] = mt_sb

                for wname, wd in (("wk", wk_d), ("wv", wv_d), ("wq", wq_d)):
                    w_sb = pers.tile(
                        [128, NDI, DL], BF16, tag=wname, name=f"{wname}_sb"
                    )
                    nc.sync.dma_start(
                        out=w_sb[:],
                        in_=wd.ap().rearrange("(c p) l -> p c l", p=128),
                    )
                    env[f"{wname}_r"] = w_sb[:]
                wo_sb = pers.tile([128, NDC, D], BF16, tag="wo", name="wo_sb")
                nc.sync.dma_start(
                    out=wo_sb[:], in_=wo_d.ap().rearrange("(c p) e -> p c e", p=128)
                )
                env["wo_r"] = wo_sb[:]

                Ksb = pers.tile([128, NDC, S], BF16, tag="Ksb", name="Ksb")
                vts = [
                    pers.tile([128, NH * 65], BF16, tag=f"vt{sc}", name=f"vt{sc}")
                    for sc in range(NSC)
                ]
                env.update(Ksb=Ksb, vts=vts)

                for qb in range(NSB):
                    env["Qblk"][qb] = qbp.tile(
                        [128, NDC, 512], BF16, tag="Qblk", name=f"Qblk{qb}"
                    )
                    env["Xblk"][qb] = xbp.tile(
                        [128, NDC, 512], BF16, tag="Xblk", name=f"Xblk{qb}"
                    )

                def load_gen_mask(qb):
                    m = mqp.tile([128, NSC, 512], BF16, tag="mq", name=f"mq{qb}")
                    nc.sync.dma_start(
                        out=m[:],
                        in_=mT_d.ap()[:, qb * 512 : (qb + 1) * 512].rearrange(
                            "(c p) q -> p c q", p=128
                        ),
                    )
                    env["mq_sb"][qb % 2] = m

                causal = variant == "causal"
                # ---- prologue: block 0 (+1) loads, K/V proj, Q proj ----
                _emit_loads(env, 0)
                if not causal:
                    for sb in range(1, NSB):
                        _emit_loads(env, sb)
                else:
                    _emit_loads(env, 1)
                if variant == "general":
                    load_gen_mask(0)
                kblocks = [0] if causal else list(range(NSB))
                if "qk" not in skip:
                    for sb in kblocks:
                        for dc in range(NDC):
                            _kproj_group(env, sb, dc)
                if "v" not in skip:
                    for sb in kblocks:
                        for sc in range(4 * sb, 4 * sb + 4):
                            _vproj_group(env, sc)
                if "qk" not in skip:
                    for dc in range(NDC):
                        _qproj_group(env, 0, dc)

                # ---- main blocks ----
                for qb in range(NSB):
                    if causal and qb + 2 < NSB:
                        _emit_loads(env, qb + 2)
                    if variant == "general" and qb + 1 < NSB:
                        load_gen_mask(qb + 1)
                    filler = []
                    nqb = qb + 1
                    if nqb < NSB:
                        if causal and "qk" not in skip:
                            for dc in range(NDC):
                                filler.append(
                                    lambda s=nqb, d=dc: _kproj_group(env, s, d)
                                )
                        if causal and "v" not in skip:
                            for sc in range(4 * nqb, 4 * nqb + 4):
                                filler.append(lambda s=sc: _vproj_group(env, s))
                        if "qk" not in skip:
                            for dc in range(NDC):
                                filler.append(
                                    lambda q=nqb, d=dc: _qproj_group(env, q, d)
                                )
                    if qb > 0 and not ({"out", "pv", "attn", "exp"} & set(skip)):
                        for ec in range(NDI):
                            filler.append(
                                lambda q=qb - 1, e=ec: _oproj_group(env, q, e)
                            )
                    if "attn" not in skip:
                        _emit_attention(env, qb, filler)
                    else:
                        for f in filler:
                            f()
                if not ({"out", "pv", "attn", "exp"} & set(skip)):
                    for ec in range(NDI):
                        _oproj_group(env, NSB - 1, ec)
    nc.compile()
    return nc


# ---------------------------------------------------------------------------
# host side
# ---------------------------------------------------------------------------

_NC_CACHE = {}


def _get_program(variant, reps=1):
    key = (variant, reps)
    if key not in _NC_CACHE:
        _NC_CACHE[key] = build_program(variant, reps)
    return _NC_CACHE[key]


def detect_variant(mask):
    m = np.asarray(mask)
    if (m != 0).all():
        return "ones"
    tril = np.tril(np.ones((S, S), np.int8))
    for b in range(m.shape[0]):
        mb = (m[b] != 0).astype(np.int8)
        if not np.array_equal(mb, tril):
            return "general"
    return "causal"


def make_causal_mask_tiles():
    j = np.arange(4)[:, None, None]
    k = np.arange(128)[None, :, None]
    q = np.arange(512)[None, None, :]
    # multiplicative: 1 keep, 0 drop (applied to exp'd scores); both halves
    m = (q >= k + 128 * j).astype(NPBF16)
    return np.concatenate([m, m], axis=2)


def build_in_maps(query, key, value, mask, Wq, bq, Wk, bk, Wv, bv, Wo, bo, variant):
    query = np.asarray(query, np.float32)
    key = np.asarray(key, np.float32)
    value = np.asarray(value, np.float32)
    Wq, Wk, Wv, Wo = (np.asarray(w, np.float32) for w in (Wq, Wk, Wv, Wo))
    bq, bk, bv = (np.asarray(x, np.float32) for x in (bq, bk, bv))

    if variant == "causal":
        mtiles = make_causal_mask_tiles()

    in_maps = []
    for c in range(8):
        b, g = c // 2, c % 2
        gs = slice(g * DL, (g + 1) * DL)
        m = {
            "qT": np.ascontiguousarray(query[b].T.astype(NPBF16)),
            "kT": np.ascontiguousarray(key[b].T.astype(NPBF16)),
            "vT": np.ascontiguousarray(value[b].T.astype(NPBF16)),
            "wq": np.ascontiguousarray(Wq[gs].T.astype(NPBF16)),
            "wk": np.ascontiguousarray(Wk[gs].T.astype(NPBF16)),
            "wv": np.ascontiguousarray(Wv[gs].T.astype(NPBF16)),
            "wo": np.ascontiguousarray(Wo[:, gs].T.astype(NPBF16)),
            "bq": np.ascontiguousarray(bq[gs].reshape(NDC, 128).T),
            "bk": np.ascontiguousarray(bk[gs].reshape(NDC, 128).T),
            "bv": np.ascontiguousarray(np.broadcast_to(bv[gs], (128, DL))),
        }
        if variant == "causal":
            m["maskt"] = mtiles
        elif variant == "general":
            m["maskT"] = np.ascontiguousarray(
                (np.asarray(mask[b]) != 0).astype(NPBF16).T
            )
        in_maps.append(m)
    return in_maps


def assemble_output(results, bo):
    bo = np.asarray(bo, np.float32)
    out = np.empty((B, S, D), np.float32)
    for b in range(B):
        acc = results[2 * b]["outT"].astype(np.float32) + results[
            2 * b + 1
        ]["outT"].astype(np.float32)
        out[b] = acc.T + bo
    return out


def kernel(query, key, value, mask, Wq, bq, Wk, bk, Wv, bv, Wo, bo):
    variant = detect_variant(np.asarray(mask))
    in_maps = build_in_maps(
        query, key, value, mask, Wq, bq, Wk, bk, Wv, bv, Wo, bo, variant
    )
    nc = _get_program(variant)
    res = run_bass_kernel_spmd(nc, in_maps, core_ids=list(range(8)))
    return assemble_output(res.results, bo)
